# revision 6
# baseline (speedup 1.0000x reference)
"""Trainium2 Bass kernel for a 2-layer dense GAT (GraphAttention) autoencoder.

reference math (per layer):
    Wh = h @ W;  e1 = Wh @ a[:F];  e2 = Wh @ a[F:]
    e  = leakyrelu(e1_i + e2_j, 0.2);  e = where(adj>0, e, -9e15)
    att = softmax(e, axis=-1);  out = elu(att @ Wh)
returns (dec, enc) = (layer2(layer1(x)), layer1(x))

Kernel strategy (8 NeuronCores, node/row parallel):
  - each core owns NL=1024 rows of adj (its output nodes i); host passes the
    row-slice per core, x replicated.
  - softmax without max-subtraction (logits are small, |z| < ~5):
    p_ij = adj_ij * exp(leaky(z_ij));  out_i = (sum_j p_ij Wh_j) / (sum_j p_ij)
  - additive mask folded into transposed fp16 tiles S0[j,i] in {0, -60000},
    built on-device from adj rows via one tensor_scalar + batched 3D-output
    DMA-xbar transposes (out[p,b,i] = in[i, b*128+p]), reused by both layers.
  - exp(leaky(z)) == max(exp(z), exp(0.2 z)) on ScalarE (free scale), or a
    DVE-side leaky + one exp for a tunable fraction of tiles (engine balance).
  - aggregation in transposed-output orientation: psum[f, i] accumulates
    lhsT=WhE[j, f-chunk], rhs=p[j, i-chunk] over j; yields enc^T directly,
    which feeds layer 2's feature matmul with no big inter-layer transpose.
    One AllGather of enc^T fp16 at the boundary. Rowsums via an extra
    ones[j,128] stationary matmul. Outputs transposed back via PE (fp32).
"""

import os
import sys

import numpy as np

for _p in ("/opt/trn_rl_repo", os.environ.get("TRN_RL_REPO", "")):
    if _p and _p not in sys.path and os.path.isdir(_p):
        sys.path.insert(0, _p)

import concourse.bass as bass
import concourse.bacc as bacc
import concourse.mybir as mybir
from concourse import tile
from concourse.masks import make_identity

F32 = mybir.dt.float32
F16 = mybir.dt.float16
AF = mybir.ActivationFunctionType
OP = mybir.AluOpType

N_CORES = 8
N_NODES = 8192
F_IN = 512
F_HID = 256
ALPHA = 0.2
MASKNEG = -60000.0

# groups (of G j-blocks) per i-chunk whose leaky-relu runs on VectorE
# instead of a second Exp on ScalarE
DVE_LEAKY_GROUPS = 4


def _ceil_div(a, b):
    return (a + b - 1) // b


def build_program(n_nodes=N_NODES, f_in=F_IN, f_hid=F_HID, s0_res=32):
    NL = n_nodes // N_CORES            # nodes per core
    JBT = n_nodes // 128               # j-blocks total
    NB = NL // 128                     # my node blocks
    IC_W = min(512, NL)                # i-chunk width for aggregation
    NIC = NL // IC_W
    G = min(8, JBT)                    # j-blocks per elementwise mega-group
    NGRP = JBT // G
    S0_RES = min(s0_res, JBT)          # SBUF-resident S0 j-blocks (megaA)
    S0_SPILL = JBT - S0_RES

    KI = f_in // 128
    KH = f_hid // 128
    FC1 = f_hid // 128
    FC2 = f_in // 128

    nc = bacc.Bacc()

    adj_rows = nc.dram_tensor("adj_rows", [NL, n_nodes], F32, kind="ExternalInput")
    x_full = nc.dram_tensor("x_full", [n_nodes, f_in], F32, kind="ExternalInput")
    x_mine = nc.dram_tensor("x_mine", [NL, f_in], F32, kind="ExternalInput")
    w1_d = nc.dram_tensor("w1", [f_in, f_hid], F32, kind="ExternalInput")
    a1_d = nc.dram_tensor("a1", [2 * f_hid, 1], F32, kind="ExternalInput")
    w2_d = nc.dram_tensor("w2", [f_hid, f_in], F32, kind="ExternalInput")
    a2_d = nc.dram_tensor("a2", [2 * f_in, 1], F32, kind="ExternalInput")

    dec_out = nc.dram_tensor("dec_out", [NL, f_in], F32, kind="ExternalOutput")
    enc_out = nc.dram_tensor("enc_out", [NL, f_hid], F32, kind="ExternalOutput")

    # spill store: per node-block ib, the staged dump [128, S0_SPILL, 128]
    if S0_SPILL:
        s0_spill_d = nc.dram_tensor("s0_spill", [NB, 128, S0_SPILL, 128], F16)
    else:
        s0_spill_d = None
    encT_my_d = nc.dram_tensor("encT_my", [f_hid, NL], F16)
    encT_full_d = nc.dram_tensor(
        "encT_full", [N_CORES * f_hid, NL], F16, addr_space="Shared"
    )

    C = dict(
        NL=NL, JBT=JBT, NB=NB, IC_W=IC_W, NIC=NIC, G=G, NGRP=NGRP,
        S0_RES=S0_RES, S0_SPILL=S0_SPILL, KI=KI, KH=KH, FC1=FC1, FC2=FC2,
        f_in=f_in, f_hid=f_hid, n_nodes=n_nodes,
    )
    with tile.TileContext(nc) as tc:
        _emit(tc, nc, C, adj_rows, x_full, x_mine, w1_d, a1_d, w2_d, a2_d,
              dec_out, enc_out, s0_spill_d, encT_my_d, encT_full_d)
    nc.finalize()
    return nc


def _emit(tc, nc, C, adj_rows, x_full, x_mine, w1_d, a1_d, w2_d, a2_d,
          dec_out, enc_out, s0_spill_d, encT_my_d, encT_full_d):
    from contextlib import ExitStack

    NL, JBT, NB = C["NL"], C["JBT"], C["NB"]
    IC_W, NIC, G, NGRP = C["IC_W"], C["NIC"], C["G"], C["NGRP"]
    S0_RES, S0_SPILL = C["S0_RES"], C["S0_SPILL"]
    KI, KH, FC1, FC2 = C["KI"], C["KH"], C["FC1"], C["FC2"]
    f_in, f_hid, n_nodes = C["f_in"], C["f_hid"], C["n_nodes"]

    est = ExitStack()
    with est:
        # ---------------- persistent pools ----------------
        s0_pool = est.enter_context(tc.tile_pool(name="s0", bufs=1))
        whe_pool = est.enter_context(tc.tile_pool(name="whe", bufs=1))
        cst_pool = est.enter_context(tc.tile_pool(name="cst", bufs=1))

        s0_megaA = s0_pool.tile([128, S0_RES, NL], F16, name="s0_megaA")
        whe_tiles = [
            whe_pool.tile([128, f_in], F16, tag=f"whe_{j}", name=f"whe_{j}")
            for j in range(JBT)
        ]
        w1e = [cst_pool.tile([128, f_hid + 2], F16, tag=f"w1e_{k}", name=f"w1e_{k}") for k in range(KI)]
        w2e = [cst_pool.tile([128, f_in + 2], F16, tag=f"w2e_{k}", name=f"w2e_{k}") for k in range(KH)]
        we1b_1 = [cst_pool.tile([128, 128], F16, tag=f"we1b1_{k}", name=f"we1b1_{k}") for k in range(KI)]
        we1c_1 = [cst_pool.tile([128, 1], F32, tag=f"we1c1_{k}", name=f"we1c1_{k}") for k in range(KI)]
        we1b_2 = [cst_pool.tile([128, 128], F16, tag=f"we1b2_{k}", name=f"we1b2_{k}") for k in range(KH)]
        we1c_2 = [cst_pool.tile([128, 1], F32, tag=f"we1c2_{k}", name=f"we1c2_{k}") for k in range(KH)]
        e2f_1 = cst_pool.tile([128, JBT], F32, tag="e2f_1", name="e2f_1")
        e2f_2 = cst_pool.tile([128, JBT], F32, tag="e2f_2", name="e2f_2")
        e1b = cst_pool.tile([128, NL], F16, tag="e1b", name="e1b")
        encT_sh = [cst_pool.tile([128, NL], F16, tag=f"encT_{c}", name=f"encT_{c}") for c in range(FC1)]
        ones_t = cst_pool.tile([128, 128], F16, tag="ones_t", name="ones_t")
        ident = cst_pool.tile([128, 128], F32, tag="ident", name="ident")

        nc.gpsimd.memset(ones_t[:], 1.0)
        make_identity(nc, ident[:])

        # ---------------- weight prep ----------------
        with (
            tc.tile_pool(name="wprep", bufs=2) as wp,
            tc.tile_pool(name="wprep_ps", bufs=2, space=bass.MemorySpace.PSUM) as wps,
        ):
            def build_we(w_d, a_d, fi, fo, kt, out_tiles, we1c_tiles):
                ka = fo // 128
                a_t = wp.tile([128, 2 * ka], F32, tag="a_t", name="a_t")
                nc.sync.dma_start(
                    a_t[:], a_d.rearrange("(a b) c -> b (a c)", b=128)
                )
                a_c = wp.tile([128, 2 * ka], F16, tag="a_c", name="a_c")
                nc.vector.tensor_copy(a_c[:], a_t[:])
                for k in range(kt):
                    wt = wp.tile([128, fo], F32, tag="wt", name="wt")
                    nc.sync.dma_start(wt[:], w_d[k * 128:(k + 1) * 128, :])
                    wc = wp.tile([128, fo], F16, tag="wc", name="wc")
                    nc.vector.tensor_copy(wc[:], wt[:])
                    nc.vector.tensor_copy(out_tiles[k][:, :fo], wc[:])
                    wtt = wp.tile([128, ka, 128], F16, tag="wtt", name="wtt")
                    nc.sync.dma_start(wtt[:], wc[:], transpose=True)
                    pse = wps.tile([128, 2], F32, tag="pse", name="pse")
                    for kk in range(ka):
                        nc.tensor.matmul(
                            pse[:], wtt[:, kk, :], a_c[:, kk::ka],
                            start=(kk == 0), stop=(kk == ka - 1),
                        )
                    nc.vector.tensor_copy(out_tiles[k][:, fo:fo + 2], pse[:])
                    nc.vector.tensor_copy(we1c_tiles[k][:], pse[:, 0:1])

            build_we(w1_d, a1_d, f_in, f_hid, KI, w1e, we1c_1)
            build_we(w2_d, a2_d, f_hid, f_in, KH, w2e, we1c_2)

            for k in range(KI):
                nc.gpsimd.memset(we1b_1[k][:], 0.0)
                nc.vector.tensor_scalar(
                    we1b_1[k][:], we1b_1[k][:], we1c_1[k][:], None, OP.add,
                )
            for k in range(KH):
                nc.gpsimd.memset(we1b_2[k][:], 0.0)
                nc.vector.tensor_scalar(
                    we1b_2[k][:], we1b_2[k][:], we1c_2[k][:], None, OP.add,
                )

        # ---------------- layer-1 Wh pass (full graph, replicated) ----------
        with (
            tc.tile_pool(name="whp", bufs=3) as xp,
            tc.tile_pool(name="whp_ps", bufs=2, space=bass.MemorySpace.PSUM) as xps,
        ):
            def x_block(src_dram, jb):
                xst = xp.tile([128, f_in], F32, tag="xst", name="xst")
                nc.sync.dma_start(xst[:], src_dram[jb * 128:(jb + 1) * 128, :])
                xc = xp.tile([128, f_in], F16, tag="xc", name="xc")
                nc.vector.tensor_copy(xc[:], xst[:])
                xT = xp.tile([128, KI, 128], F16, tag="xT", name="xT")
                nc.sync.dma_start(xT[:], xc[:], transpose=True)
                return xT

            for jb in range(JBT):
                xT = x_block(x_full, jb)
                pswh = xps.tile([128, f_hid + 2], F32, tag="pswh", name="pswh")
                for k in range(KI):
                    nc.tensor.matmul(
                        pswh[:], xT[:, k, :], w1e[k][:],
                        start=(k == 0), stop=(k == KI - 1),
                    )
                nc.vector.tensor_copy(whe_tiles[jb][:, :f_hid], pswh[:, :f_hid])
                nc.vector.tensor_copy(
                    e2f_1[:, jb:jb + 1], pswh[:, f_hid + 1:f_hid + 2]
                )

            for nb in range(NB):
                xT = x_block(x_mine, nb)
                pse1 = xps.tile([128, 128], F32, tag="pse1", name="pse1")
                for k in range(KI):
                    nc.tensor.matmul(
                        pse1[:], we1b_1[k][:], xT[:, k, :],
                        start=(k == 0), stop=(k == KI - 1),
                    )
                nc.vector.tensor_copy(e1b[:, nb * 128:(nb + 1) * 128], pse1[:])

        # ---------------- S0 prep: masked transposed adj ----------
        # quarters of 2048 adj columns; quarters < q_res land in the resident
        # mega tile, the rest in a staging tile dumped to DRAM per node-block.
        QW = min(2048, n_nodes)
        n_q = n_nodes // QW
        q_res = S0_RES * 128 // QW
        with (
            tc.tile_pool(name="sprep", bufs=2) as sp,
            tc.tile_pool(name="sstage", bufs=2) as ssg,
        ):
            for ib in range(NB):
                stageC = (
                    ssg.tile([128, S0_SPILL, 128], F16, tag="stageC", name="stageC")
                    if S0_SPILL else None
                )
                for q in range(n_q):
                    ast = sp.tile([128, QW], F32, tag="ast", name="ast")
                    nc.sync.dma_start(
                        ast[:],
                        adj_rows[ib * 128:(ib + 1) * 128, q * QW:(q + 1) * QW],
                    )
                    sst = sp.tile([128, QW], F16, tag="sst", name="sst")
                    nc.vector.tensor_scalar(
                        sst[:], ast[:], -MASKNEG, MASKNEG, OP.mult, OP.add
                    )
                    nqb = QW // 128
                    if q < q_res:
                        dst = s0_megaA[:, q * nqb:(q + 1) * nqb,
                                       ib * 128:(ib + 1) * 128]
                    else:
                        qq = q - q_res
                        dst = stageC[:, qq * nqb:(qq + 1) * nqb, :]
                    nc.sync.dma_start(dst, sst[:], transpose=True)
                if S0_SPILL:
                    nc.sync.dma_start(s0_spill_d[ib], stageC[:])

        # ---------------- the attention layer emitter ----------------
        def agg_layer(whe_f, fc, e2f, spool, spsum, out_emit):
            for ic in range(NIC):
                psA = [
                    spsum.tile([128, IC_W], F32, tag=f"psA{c}", name=f"psA{c}")
                    for c in range(fc)
                ]
                psS = spsum.tile([128, IC_W], F32, tag="psS", name="psS")
                for grp in range(NGRP):
                    zB = spool.tile([128, G * IC_W], F16, tag="zB", name="zB")
                    uM = spool.tile([128, G * IC_W], F16, tag="uM", name="uM")
                    for g in range(G):
                        jb = grp * G + g
                        sl = slice(g * IC_W, (g + 1) * IC_W)
                        if jb < S0_RES:
                            s0src = s0_megaA[:, jb, ic * IC_W:(ic + 1) * IC_W]
                        else:
                            stile = spool.tile(
                                [128, IC_W], F16, tag="s0ld", name="s0ld", bufs=4
                            )
                            js = jb - S0_RES
                            nc.sync.dma_start(
                                stile[:].rearrange("p (b i) -> p b i", i=128),
                                s0_spill_d[ic * (IC_W // 128):(ic + 1) * (IC_W // 128),
                                           :, js:js + 1, :].rearrange(
                                               "b p j i -> p (b j) i"),
                            )
                            s0src = stile[:]
                        nc.vector.tensor_scalar(
                            zB[:, sl], s0src, e2f[:, jb:jb + 1], None, OP.add
                        )
                        nc.vector.tensor_tensor(
                            zB[:, sl], zB[:, sl],
                            e1b[:, ic * IC_W:(ic + 1) * IC_W], OP.add,
                        )
                    if grp < DVE_LEAKY_GROUPS:
                        nc.vector.tensor_scalar(uM[:], zB[:], ALPHA, None, OP.mult)
                        nc.vector.tensor_tensor(zB[:], zB[:], uM[:], OP.max)
                        nc.scalar.activation(uM[:], zB[:], AF.Exp)
                    else:
                        nc.scalar.activation(uM[:], zB[:], AF.Exp)
                        nc.scalar.activation(zB[:], zB[:], AF.Exp, scale=ALPHA)
                        nc.vector.tensor_tensor(uM[:], uM[:], zB[:], OP.max)
                    for g in range(G):
                        jb = grp * G + g
                        sl = slice(g * IC_W, (g + 1) * IC_W)
                        first, last = (jb == 0), (jb == JBT - 1)
                        wsrc = whe_f(jb)
                        for c in range(fc):
                            nc.tensor.matmul(
                                psA[c][:], wsrc[:, c * 128:(c + 1) * 128],
                                uM[:, sl], start=first, stop=last,
                            )
                        nc.tensor.matmul(
                            psS[:], ones_t[:], uM[:, sl],
                            start=first, stop=last,
                        )
                rec = spool.tile([128, IC_W], F32, tag="rec", name="rec")
                nc.vector.reciprocal(rec[:], psS[:])
                for c in range(fc):
                    onrm = spool.tile([128, IC_W], F32, tag="onrm", name="onrm")
                    nc.vector.tensor_tensor(onrm[:], psA[c][:], rec[:], OP.mult)
                    tmin = spool.tile([128, IC_W], F32, tag="tmin", name="tmin")
                    nc.vector.tensor_scalar(tmin[:], onrm[:], 0.0, None, OP.min)
                    emt = spool.tile([128, IC_W], F32, tag="emt", name="emt")
                    nc.scalar.activation(emt[:], tmin[:], AF.Exp)
                    nc.vector.tensor_scalar(onrm[:], onrm[:], 0.0, None, OP.max)
                    nc.vector.tensor_tensor(onrm[:], onrm[:], emt[:], OP.add)
                    nc.vector.tensor_scalar(onrm[:], onrm[:], -1.0, None, OP.add)
                    out_emit(ic, c, onrm, spool, spsum)

        # ---------------- layer 1 ----------------
        def l1_emit(ic, c, onrm, spool, spsum):
            nc.vector.tensor_copy(
                encT_sh[c][:, ic * IC_W:(ic + 1) * IC_W], onrm[:]
            )
            for t in range(IC_W // 128):
                pst = spsum.tile([128, 128], F32, tag="pst", name="pst")
                nc.tensor.transpose(
                    pst[:], onrm[:, t * 128:(t + 1) * 128], ident[:]
                )
                stg = spool.tile([128, 128], F32, tag="ostg", name="ostg", bufs=2)
                nc.scalar.copy(stg[:], pst[:])
                r0 = ic * IC_W + t * 128
                nc.sync.dma_start(
                    enc_out[r0:r0 + 128, c * 128:(c + 1) * 128], stg[:]
                )

        with (
            tc.tile_pool(name="c1", bufs=2) as c1p,
            tc.tile_pool(name="c1ps", bufs=1, space=bass.MemorySpace.PSUM) as c1ps,
        ):
            agg_layer(
                lambda jb: whe_tiles[jb][:, :f_hid], FC1, e2f_1, c1p, c1ps,
                l1_emit,
            )

        # ---------------- allgather enc^T ----------------
        for c in range(FC1):
            nc.sync.dma_start(
                encT_my_d[c * 128:(c + 1) * 128, :], encT_sh[c][:]
            )
        nc.gpsimd.collective_compute(
            "AllGather", OP.bypass,
            replica_groups=[list(range(N_CORES))],
            ins=[encT_my_d[:, :]],
            outs=[encT_full_d[:, :]],
        )

        # ---------------- layer-2 Wh pass ----------------
        with (
            tc.tile_pool(name="wh2", bufs=3) as w2p,
            tc.tile_pool(name="wh2_ps", bufs=2, space=bass.MemorySpace.PSUM) as w2ps,
        ):
            nb_per_rank = NL // 128
            for jb in range(JBT):
                r, nbp = jb // nb_per_rank, jb % nb_per_rank
                psa = w2ps.tile([128, f_in], F32, tag="psa", name="psa")
                psb = w2ps.tile([128, 2], F32, tag="psb", name="psb")
                for k in range(KH):
                    eT = w2p.tile([128, 128], F16, tag=f"eT{k}", name=f"eT{k}")
                    nc.sync.dma_start(
                        eT[:],
                        encT_full_d[r * f_hid + k * 128:r * f_hid + (k + 1) * 128,
                                    nbp * 128:(nbp + 1) * 128],
                    )
                    nc.tensor.matmul(
                        psa[:], eT[:], w2e[k][:, :f_in],
                        start=(k == 0), stop=(k == KH - 1),
                    )
                    nc.tensor.matmul(
                        psb[:], eT[:], w2e[k][:, f_in:f_in + 2],
                        start=(k == 0), stop=(k == KH - 1),
                    )
                nc.vector.tensor_copy(whe_tiles[jb][:], psa[:])
                nc.vector.tensor_copy(e2f_2[:, jb:jb + 1], psb[:, 1:2])

            for h in range(_ceil_div(NL, 512)):
                w = min(512, NL - h * 512)
                pse1 = w2ps.tile([128, 512], F32, tag="pse1", name="pse1")
                for k in range(KH):
                    nc.tensor.matmul(
                        pse1[:, :w], we1b_2[k][:],
                        encT_sh[k][:, h * 512:h * 512 + w],
                        start=(k == 0), stop=(k == KH - 1),
                    )
                nc.vector.tensor_copy(e1b[:, h * 512:h * 512 + w], pse1[:, :w])

        # ---------------- layer 2 ----------------
        def l2_emit(ic, c, onrm, spool, spsum):
            for t in range(IC_W // 128):
                pst = spsum.tile([128, 128], F32, tag="pst", name="pst")
                nc.tensor.transpose(
                    pst[:], onrm[:, t * 128:(t + 1) * 128], ident[:]
                )
                stg = spool.tile([128, 128], F32, tag="ostg", name="ostg", bufs=2)
                nc.scalar.copy(stg[:], pst[:])
                r0 = ic * IC_W + t * 128
                nc.sync.dma_start(
                    dec_out[r0:r0 + 128, c * 128:(c + 1) * 128], stg[:]
                )

        with (
            tc.tile_pool(name="c2", bufs=2) as c2p,
            tc.tile_pool(name="c2ps", bufs=1, space=bass.MemorySpace.PSUM) as c2ps,
        ):
            agg_layer(
                lambda jb: whe_tiles[jb][:], FC2, e2f_2, c2p, c2ps, l2_emit,
            )


_PROGRAM_CACHE = {}


def _get_program(n_nodes, f_in, f_hid):
    key = (n_nodes, f_in, f_hid)
    if key not in _PROGRAM_CACHE:
        _PROGRAM_CACHE[key] = build_program(n_nodes, f_in, f_hid)
    return _PROGRAM_CACHE[key]


def kernel(x, adj, W1, a1, W2, a2):
    from concourse.bass_utils import run_bass_kernel_spmd

    x = np.ascontiguousarray(np.asarray(x, dtype=np.float32))
    adj = np.ascontiguousarray(np.asarray(adj, dtype=np.float32))
    W1 = np.ascontiguousarray(np.asarray(W1, dtype=np.float32))
    a1 = np.ascontiguousarray(np.asarray(a1, dtype=np.float32))
    W2 = np.ascontiguousarray(np.asarray(W2, dtype=np.float32))
    a2 = np.ascontiguousarray(np.asarray(a2, dtype=np.float32))

    n_nodes, f_in = x.shape
    f_hid = W1.shape[1]
    NL = n_nodes // N_CORES

    nc = _get_program(n_nodes, f_in, f_hid)

    in_maps = []
    for c in range(N_CORES):
        rows = slice(c * NL, (c + 1) * NL)
        in_maps.append({
            "adj_rows": np.ascontiguousarray(adj[rows, :]),
            "x_full": x,
            "x_mine": np.ascontiguousarray(x[rows, :]),
            "w1": W1, "a1": a1, "w2": W2, "a2": a2,
        })

    res = run_bass_kernel_spmd(nc, in_maps, core_ids=list(range(N_CORES)))
    dec = np.concatenate([res.results[c]["dec_out"] for c in range(N_CORES)], axis=0)
    enc = np.concatenate([res.results[c]["enc_out"] for c in range(N_CORES)], axis=0)
    return dec, enc


if __name__ == "__main__":
    import reference as R

    inputs = R.setup_inputs()
    dec, enc = kernel(**{k: np.asarray(v) for k, v in inputs.items()})
    print(dec.shape, enc.shape)
